# revision 25
# baseline (speedup 1.0000x reference)
"""AWQ linear (int4 group-quantized) matmul on 8 Trainium2 NeuronCores.

out[m, n] = sum_k x[m, k] * W[n, k] + bias[n]
W[n, k] = (q4[n, k] - qzeros[n, k//128]) * qscales[n, k//128]

Column-parallel: shard N=11008 across 8 cores (1376 each), replicate x.
The weights are dequantized to bf16 on the host (same DMA bytes as the
previous device-side scheme, which already shipped the nibbles as bf16)
and streamed straight into resident SBUF, so the PE starts as soon as
the first k-tiles land and the DVE only does psum evictions:
  - host computes W^T bf16 [K, N], partition-major per k-tile, plus the
    x^T swizzle so every (k-group, m-tile) slab is one contiguous DMA
  - m-tile block 0 accumulates over k in splits of 2/2/4/8/16 k-tiles
    so the PE starts after the first 2 W k-tiles (~2us of DMA); later
    splits' W loads are issued on the Sync queue behind the x slabs so
    x-pool backpressure paces them just-in-time (HBM reads are the
    scarce resource while W streams in)
  - the remaining m-tiles run one full-k accumulation per psum chunk
  - bias is fused into the first eviction; the last m-tile ships its
    output per-chunk to shorten the kernel tail
"""

import os

import numpy as np
import ml_dtypes

M, K, NFULL = 4096, 4096, 11008
NCORES = 8
NS = NFULL // NCORES          # 1376 out-features per core
P = 128                       # partitions; also the quant group size
MM_FREE = 512                 # psum bank limit (fp32)
XG = 2                        # k-tiles per x-slab group in the host layout

LAST_RESULTS = None           # BassKernelResults of the last kernel() call


def build_nc(k=K, m=M, ns=NS, n_cores=NCORES, splits=(2, 2, 4, 8, 16), mt_block=8, xg=XG):
    """Build + compile the per-core Bass program (SPMD: same NEFF on all cores).

    Block 0 accumulates over k in `splits` (so the PE can start while later
    k-groups' weights are still in flight); the remaining blocks run one
    full-k span.
    """
    import concourse.bass as bass
    import concourse.mybir as mybir
    import concourse.tile as tile
    from concourse import bacc

    kt_n = k // P
    mt_n = m // P
    assert sum(splits) == kt_n and mt_block <= mt_n
    assert all(s % xg == 0 for s in splits) and all(s % 2 == 0 for s in splits)
    chunks = [(i, min(MM_FREE, ns - i)) for i in range(0, ns, MM_FREE)]

    f32 = mybir.dt.float32
    bf16 = mybir.dt.bfloat16
    ADD = mybir.AluOpType.add

    nc = bacc.Bacc("TRN2", num_devices=n_cores)
    # xt rows are (kg, mt, p): each (kg, mt) slab is contiguous [128, xg*128]
    xt = nc.dram_tensor("xt", [(kt_n // xg) * mt_n * P, xg * P], bf16, kind="ExternalInput")
    # host-dequantized W^T, rows are partitions: wd[p, kt*ns + n] = W^T[kt*128 + p, n]
    wd = nc.dram_tensor("wd", [P, kt_n * ns], bf16, kind="ExternalInput")
    # bias ships bf16 (halves the partition-broadcast bytes competing with the
    # critical first W k-tile); converted once to fp32 on-chip
    bias = nc.dram_tensor("bias", [1, ns], bf16, kind="ExternalInput")
    # out ships bf16 (halves write traffic + the kernel-tail DMA);
    # the host converts back to fp32 after the gather
    out = nc.dram_tensor("out", [m, ns], bf16, kind="ExternalOutput")

    with tile.TileContext(nc) as tc:
        with (
            tc.tile_pool(name="persist", bufs=1) as persist,
            tc.tile_pool(name="xp0", bufs=3) as xp0,
            tc.tile_pool(name="xp1", bufs=4) as xp1,
            tc.tile_pool(name="xp2", bufs=3) as xp2,
            tc.tile_pool(name="op0", bufs=mt_block) as op0,
            tc.tile_pool(name="opb", bufs=4) as opb,
            tc.tile_pool(name="ps", bufs=8, space="PSUM") as ps,
        ):
            w_all = persist.tile([P, kt_n, ns], bf16)
            bias_exp = persist.tile([P, ns], f32)
            bias_bf = persist.tile([P, ns], bf16)

            def load_pair(i, eng=None):
                """DMA W k-tiles 2i and 2i+1 into resident SBUF (one load
                covers both tiles -- contiguous per partition). Early pairs
                ride the SWDGE ring; pairs interleaved into block 0 ride the
                Sync queue BEHIND the x slabs, so x-pool backpressure paces
                the W stream just-in-time instead of letting it saturate HBM
                reads and starve the x path."""
                eng = eng or nc.sync
                kt = 2 * i
                if i == 0:
                    # pair 0 gates the first sweeps: load per (k-tile,
                    # n-chunk) in sweep order, k-tile 0 on the SWDGE ring and
                    # k-tile 1 on the ACT ring, so the very first matmul
                    # waits on a single 128KB transfer
                    for n0, szc in chunks:
                        for j, eng_j in ((0, nc.gpsimd), (1, nc.scalar)):
                            srcap = bass.AP(
                                wd.ap().tensor,
                                j * ns + n0,
                                [[kt_n * ns, P], [1, szc]],
                            )
                            eng_j.dma_start(w_all[:, j, n0:n0 + szc], srcap)
                    return
                src = wd.ap()[:, kt * ns:(kt + 2) * ns]
                eng.dma_start(
                    w_all[:, kt:kt + 2, :],
                    src.rearrange("p (j n) -> p j n", n=ns),
                )

            def x_slab(g0, ng, mt, first=False):
                """Load x k-groups g0..g0+ng-1 for m-tile mt: [128, ng, xg*128],
                as ONE strided DMA (one issue slot on the Sync queue). si=0
                uses a dedicated depth-2 pool: its third slab (and the W pair
                queued behind it) then can't issue until the PE is running,
                keeping HBM clear for the startup-critical transfers."""
                pool = xp0 if first else (xp1 if ng == 1 else xp2)
                xbf = pool.tile([P, ng, xg * P], bf16,
                                tag=f"xbf{ng}a" if first else f"xbf{ng}")
                row = xg * P
                base = (g0 * mt_n + mt) * P
                src = bass.AP(
                    xt.ap().tensor,
                    base * row,
                    [[row, P], [mt_n * P * row, ng], [1, row]],
                )
                nc.sync.dma_start(xbf[:], src)
                return xbf

            def mm_sweep(pst, sz_args, kt0, n_kt, slabs, slab_kts):
                """Accumulate kt0..kt0+n_kt-1 into pst from the given x slabs."""
                nstart, sz = sz_args
                for kl in range(n_kt):
                    kt = kt0 + kl
                    sb_i = next(i for i, (a, b) in enumerate(slab_kts) if a <= kt < b)
                    loc = kt - slab_kts[sb_i][0]
                    nc.tensor.matmul(
                        pst[:, :sz],
                        slabs[sb_i][:, loc // xg, (loc % xg) * P:(loc % xg + 1) * P],
                        w_all[:, kt, nstart:nstart + sz],
                        start=(kl == 0),
                        stop=(kl == n_kt - 1),
                    )

            for i in range(splits[0] // 2):
                load_pair(i, eng=nc.gpsimd)
            # bias rides the ACT queue behind pair 0's k-tile-1 chunks: it is
            # only needed at the first eviction, which the psum pool lets lag
            nc.scalar.dma_start(bias_bf[:], bias.ap().to_broadcast((P, ns)))
            nc.vector.tensor_scalar_add(bias_exp[:], bias_bf[:], 0.0)

            s_n = len(splits)
            s_start = [sum(splits[:i]) for i in range(s_n)]

            # ---- block 0: k-split sweeps, W loads interleaved ----
            outsb = {}
            for si in range(s_n):
                pending = (
                    list(range(s_start[si + 1] // 2,
                               (s_start[si + 1] + splits[si + 1]) // 2))
                    if si + 1 < s_n
                    else []
                )
                # si=0: hold pending W issues until mi=3 -- behind the first
                # pool-gated x slab (xp0 depth 3), so they can't issue before
                # the PE is running
                start_mi = 3 if si == 0 else 0
                navail = mt_block - start_mi
                per_mi = (len(pending) + navail - 1) // navail if pending else 0
                for mi in range(mt_block):
                    mt = mi
                    xbf = x_slab(s_start[si] // xg, splits[si] // xg, mt,
                                 first=(si == 0))
                    span = (s_start[si], s_start[si] + splits[si])
                    if si == 0:
                        outsb[mi] = op0.tile(
                            [P, ns], f32, tag="outsb", name=f"outsb_0_{mi}"
                        )
                    if si == s_n - 1:
                        # last split: the accumulate writes a bf16 tile (the
                        # out dram tensor is bf16 -- DMA does not convert)
                        osb_bf = opb.tile(
                            [P, ns], bf16, tag="outsbf", name=f"outsbf_0_{mi}"
                        )
                    for nstart, sz in chunks:
                        pst = ps.tile([P, MM_FREE], f32, tag="psum")
                        mm_sweep(pst, (nstart, sz), span[0], splits[si], [xbf], [span])
                        osl = outsb[mi][:, nstart:nstart + sz]
                        if si == 0:
                            nc.vector.tensor_tensor(
                                osl, pst[:, :sz], bias_exp[:, nstart:nstart + sz], ADD
                            )
                        elif si == s_n - 1:
                            nc.vector.tensor_tensor(
                                osb_bf[:, nstart:nstart + sz], osl, pst[:, :sz], ADD
                            )
                        else:
                            nc.vector.tensor_tensor(osl, osl, pst[:, :sz], ADD)
                    if mi >= start_mi:
                        j = mi - start_mi
                        for i in pending[j * per_mi:(j + 1) * per_mi]:
                            load_pair(i)
                    if si == s_n - 1:
                        nc.scalar.dma_start(
                            out.ap()[mt * P:(mt + 1) * P, :], osb_bf[:]
                        )

            # ---- blocks 1+: full-k accumulation spans ----
            for mt in range(mt_block, mt_n):
                slabs = [x_slab(0, kt_n // xg, mt)]
                slab_kts = [(0, kt_n)]
                osb = opb.tile([P, ns], bf16, tag="outsbf", name=f"outsb_{mt}")
                for nstart, sz in chunks:
                    pst = ps.tile([P, MM_FREE], f32, tag="psum")
                    mm_sweep(pst, (nstart, sz), 0, kt_n, slabs, slab_kts)
                    nc.vector.tensor_tensor(
                        osb[:, nstart:nstart + sz],
                        pst[:, :sz],
                        bias_exp[:, nstart:nstart + sz],
                        ADD,
                    )
                    if mt == mt_n - 1:
                        # last m-tile: ship each chunk as soon as it lands so
                        # the kernel tail isn't one big serial DMA
                        nc.scalar.dma_start(
                            out.ap()[mt * P:(mt + 1) * P, nstart:nstart + sz],
                            osb[:, nstart:nstart + sz],
                        )
                if mt != mt_n - 1:
                    nc.scalar.dma_start(out.ap()[mt * P:(mt + 1) * P, :], osb[:])

    nc.compile()
    return nc


def prep_x(x, xg=XG):
    """bf16 x^T swizzled so each (kg, mt) slab is one contiguous [128, xg*128]
    row-block: xt[(kg*mt_n + mt)*128 + p, kl*128 + j] = x[mt*128 + j, (kg*xg + kl)*128 + p]
    """
    m, k = x.shape
    kt_n, mt_n = k // P, m // P
    kg_n = kt_n // xg
    xbf = x.astype(ml_dtypes.bfloat16)
    # [mt, j, kg, kl, p] -> [kg, mt, p, kl, j]
    xs = xbf.reshape(mt_n, P, kg_n, xg, P).transpose(2, 0, 4, 3, 1)
    return np.ascontiguousarray(xs.reshape(kg_n * mt_n * P, xg * P))


def prep_inputs(x, qweight, qscales, qzeros, bias):
    """Host-side shard/layout prep. Returns per-core input maps."""
    x = np.asarray(x)
    qweight = np.asarray(qweight)
    qscales = np.asarray(qscales).astype(np.float32)
    qzeros = np.asarray(qzeros).astype(np.float32)
    bias = np.asarray(bias)

    xprep = prep_x(x)

    # Unpack int4 nibbles into k-major fp32 [K, N], dequantize per group,
    # round once to bf16: even k -> low nibble, odd k -> high nibble.
    b = qweight.astype(np.uint8)              # [N, K//2]
    q4 = np.empty((K, NFULL), np.float32)
    q4[0::2, :] = (b & 15).T
    q4[1::2, :] = (b >> 4).T
    kt_n = K // P
    # per-group affine along k: group g covers rows [g*128, (g+1)*128)
    q4 = q4.reshape(kt_n, P, NFULL)
    q4 -= qzeros.T[:, None, :]
    q4 *= qscales.T[:, None, :]
    wbf = q4.astype(ml_dtypes.bfloat16)       # [kt, p, N]
    # partition-major: wdp[p, kt, n] = W^T[kt*128 + p, n]
    wdp = np.ascontiguousarray(wbf.transpose(1, 0, 2))

    bias2d = bias.astype(ml_dtypes.bfloat16).reshape(1, NFULL)

    in_maps = []
    for c in range(NCORES):
        sl = slice(c * NS, (c + 1) * NS)
        in_maps.append(
            {
                "xt": xprep,
                "wd": np.ascontiguousarray(wdp[:, :, sl]).reshape(P, kt_n * NS),
                "bias": np.ascontiguousarray(bias2d[:, sl]),
            }
        )
    return in_maps


def _run(in_maps, trace):
    global LAST_RESULTS
    from concourse.bass_utils import run_bass_kernel_spmd

    nc = build_nc()
    res = run_bass_kernel_spmd(
        nc,
        in_maps,
        core_ids=list(range(NCORES)),
        trace=trace,
        trace_cores=list(range(NCORES)) if trace else None,
    )
    LAST_RESULTS = res
    return np.concatenate(
        [res.results[c]["out"] for c in range(NCORES)], axis=1
    ).astype(np.float32)


def _subprocess_entry(tmpdir):
    """Retry entry point: runs the kernel in a fresh process (clean device
    state) using inputs staged in tmpdir, writes the output there."""
    import numpy as np  # noqa: F811 (fresh interpreter)

    data = np.load(os.path.join(tmpdir, "inputs.npz"))
    out = kernel(**{k: data[k] for k in data.files})
    np.save(os.path.join(tmpdir, "out.npy"), out)


def kernel(x, qweight, qscales, qzeros, bias):
    """Full-input entry: shard, run on 8 cores, gather. Retries on transient
    device failures (NRT exec-unit errors have been observed sporadically on
    this fabric): once in-process with NEURON_RT_RESET_CORES=1, then once in
    a fresh subprocess."""
    in_maps = prep_inputs(x, qweight, qscales, qzeros, bias)
    trace = bool(os.environ.get("BASS_AWQ_TRACE"))
    try:
        return _run(in_maps, trace)
    except Exception:
        pass
    os.environ["NEURON_RT_RESET_CORES"] = "1"
    try:
        return _run(in_maps, trace)
    except Exception:
        if os.environ.get("AWQ_NO_SUBPROC"):
            raise
    # last resort: fresh process (clean runtime/device handles)
    import subprocess
    import sys
    import tempfile

    with tempfile.TemporaryDirectory() as td:
        np.savez(
            os.path.join(td, "inputs.npz"),
            x=x, qweight=qweight, qscales=qscales, qzeros=qzeros, bias=bias,
        )
        code = (
            "import importlib.util, sys;"
            f"spec = importlib.util.spec_from_file_location('awq_kernel', {__file__!r});"
            "m = importlib.util.module_from_spec(spec);"
            "spec.loader.exec_module(m);"
            f"m._subprocess_entry({td!r})"
        )
        subprocess.run(
            [sys.executable, "-c", code],
            check=True,
            env={**os.environ, "NEURON_RT_RESET_CORES": "1",
                 "AWQ_NO_SUBPROC": "1"},
        )
        return np.load(os.path.join(td, "out.npy"))


# revision 26
# speedup vs baseline: 1.0095x; 1.0095x over previous
"""AWQ linear (int4 group-quantized) matmul on 8 Trainium2 NeuronCores.

out[m, n] = sum_k x[m, k] * W[n, k] + bias[n]
W[n, k] = (q4[n, k] - qzeros[n, k//128]) * qscales[n, k//128]

Column-parallel: shard N=11008 across 8 cores (1376 each), replicate x.
The weights are dequantized to bf16 on the host (same DMA bytes as the
previous device-side scheme, which already shipped the nibbles as bf16)
and streamed straight into resident SBUF, so the PE starts as soon as
the first k-tiles land and the DVE only does psum evictions:
  - host computes W^T bf16 [K, N], partition-major per k-tile, plus the
    x^T swizzle so every (k-group, m-tile) slab is one contiguous DMA
  - m-tile block 0 accumulates over k in splits of 2/2/4/8/16 k-tiles
    so the PE starts after the first 2 W k-tiles (~2us of DMA); later
    splits' W loads are issued on the Sync queue behind the x slabs so
    x-pool backpressure paces them just-in-time (HBM reads are the
    scarce resource while W streams in)
  - the remaining m-tiles run one full-k accumulation per psum chunk
  - bias is fused into the first eviction; the last m-tile ships its
    output per-chunk to shorten the kernel tail
"""

import os

import numpy as np
import ml_dtypes

M, K, NFULL = 4096, 4096, 11008
NCORES = 8
NS = NFULL // NCORES          # 1376 out-features per core
P = 128                       # partitions; also the quant group size
MM_FREE = 512                 # psum bank limit (fp32)
XG = 2                        # k-tiles per x-slab group in the host layout

LAST_RESULTS = None           # BassKernelResults of the last kernel() call


def build_nc(k=K, m=M, ns=NS, n_cores=NCORES, splits=(2, 2, 4, 8, 16), mt_block=8, xg=XG):
    """Build + compile the per-core Bass program (SPMD: same NEFF on all cores).

    Block 0 accumulates over k in `splits` (so the PE can start while later
    k-groups' weights are still in flight); the remaining blocks run one
    full-k span.
    """
    import concourse.bass as bass
    import concourse.mybir as mybir
    import concourse.tile as tile
    from concourse import bacc

    kt_n = k // P
    mt_n = m // P
    assert sum(splits) == kt_n and mt_block <= mt_n
    assert all(s % xg == 0 for s in splits) and all(s % 2 == 0 for s in splits)
    chunks = [(i, min(MM_FREE, ns - i)) for i in range(0, ns, MM_FREE)]

    f32 = mybir.dt.float32
    bf16 = mybir.dt.bfloat16
    ADD = mybir.AluOpType.add

    nc = bacc.Bacc("TRN2", num_devices=n_cores)
    # xt rows are (kg, mt, p): each (kg, mt) slab is contiguous [128, xg*128]
    xt = nc.dram_tensor("xt", [(kt_n // xg) * mt_n * P, xg * P], bf16, kind="ExternalInput")
    # host-dequantized W^T, rows are partitions: wd[p, kt*ns + n] = W^T[kt*128 + p, n]
    wd = nc.dram_tensor("wd", [P, kt_n * ns], bf16, kind="ExternalInput")
    # bias ships bf16 (halves the partition-broadcast bytes competing with the
    # critical first W k-tile); converted once to fp32 on-chip
    bias = nc.dram_tensor("bias", [1, ns], bf16, kind="ExternalInput")
    out = nc.dram_tensor("out", [m, ns], f32, kind="ExternalOutput")

    with tile.TileContext(nc) as tc:
        with (
            tc.tile_pool(name="persist", bufs=1) as persist,
            tc.tile_pool(name="xp0", bufs=3) as xp0,
            tc.tile_pool(name="xp1", bufs=4) as xp1,
            tc.tile_pool(name="xp2", bufs=4) as xp2,
            tc.tile_pool(name="op", bufs=mt_block + 1) as op,
            tc.tile_pool(name="ps", bufs=8, space="PSUM") as ps,
        ):
            w_all = persist.tile([P, kt_n, ns], bf16)
            bias_exp = persist.tile([P, ns], f32)
            bias_bf = persist.tile([P, ns], bf16)

            def load_pair(i, eng=None):
                """DMA W k-tiles 2i and 2i+1 into resident SBUF (one load
                covers both tiles -- contiguous per partition). Early pairs
                ride the SWDGE ring; pairs interleaved into block 0 ride the
                Sync queue BEHIND the x slabs, so x-pool backpressure paces
                the W stream just-in-time instead of letting it saturate HBM
                reads and starve the x path."""
                eng = eng or nc.sync
                kt = 2 * i
                if i == 0:
                    # pair 0 gates the first sweeps: load per (k-tile,
                    # n-chunk) in sweep order, k-tile 0 on the SWDGE ring and
                    # k-tile 1 on the ACT ring, so the very first matmul
                    # waits on a single 128KB transfer
                    for n0, szc in chunks:
                        for j, eng_j in ((0, nc.gpsimd), (1, nc.scalar)):
                            srcap = bass.AP(
                                wd.ap().tensor,
                                j * ns + n0,
                                [[kt_n * ns, P], [1, szc]],
                            )
                            eng_j.dma_start(w_all[:, j, n0:n0 + szc], srcap)
                    return
                src = wd.ap()[:, kt * ns:(kt + 2) * ns]
                eng.dma_start(
                    w_all[:, kt:kt + 2, :],
                    src.rearrange("p (j n) -> p j n", n=ns),
                )

            def x_slab(g0, ng, mt, first=False):
                """Load x k-groups g0..g0+ng-1 for m-tile mt: [128, ng, xg*128],
                as ONE strided DMA (one issue slot on the Sync queue). si=0
                uses a dedicated depth-2 pool: its third slab (and the W pair
                queued behind it) then can't issue until the PE is running,
                keeping HBM clear for the startup-critical transfers."""
                pool = xp0 if first else (xp1 if ng == 1 else xp2)
                xbf = pool.tile([P, ng, xg * P], bf16,
                                tag=f"xbf{ng}a" if first else f"xbf{ng}")
                row = xg * P
                base = (g0 * mt_n + mt) * P
                src = bass.AP(
                    xt.ap().tensor,
                    base * row,
                    [[row, P], [mt_n * P * row, ng], [1, row]],
                )
                nc.sync.dma_start(xbf[:], src)
                return xbf

            def mm_sweep(pst, sz_args, kt0, n_kt, slabs, slab_kts):
                """Accumulate kt0..kt0+n_kt-1 into pst from the given x slabs."""
                nstart, sz = sz_args
                for kl in range(n_kt):
                    kt = kt0 + kl
                    sb_i = next(i for i, (a, b) in enumerate(slab_kts) if a <= kt < b)
                    loc = kt - slab_kts[sb_i][0]
                    nc.tensor.matmul(
                        pst[:, :sz],
                        slabs[sb_i][:, loc // xg, (loc % xg) * P:(loc % xg + 1) * P],
                        w_all[:, kt, nstart:nstart + sz],
                        start=(kl == 0),
                        stop=(kl == n_kt - 1),
                    )

            for i in range(splits[0] // 2):
                load_pair(i, eng=nc.gpsimd)
            # bias rides the ACT queue behind pair 0's k-tile-1 chunks: it is
            # only needed at the first eviction, which the psum pool lets lag
            nc.scalar.dma_start(bias_bf[:], bias.ap().to_broadcast((P, ns)))
            nc.vector.tensor_scalar_add(bias_exp[:], bias_bf[:], 0.0)

            s_n = len(splits)
            s_start = [sum(splits[:i]) for i in range(s_n)]

            # ---- block 0: k-split sweeps, W loads interleaved ----
            outsb = {}
            for si in range(s_n):
                pending = (
                    list(range(s_start[si + 1] // 2,
                               (s_start[si + 1] + splits[si + 1]) // 2))
                    if si + 1 < s_n
                    else []
                )
                # si=0: hold pending W issues until mi=2 -- behind the first
                # pool-gated x slab, so they can't issue before the PE runs
                start_mi = 2 if si == 0 else 0
                navail = mt_block - start_mi
                per_mi = (len(pending) + navail - 1) // navail if pending else 0
                for mi in range(mt_block):
                    mt = mi
                    xbf = x_slab(s_start[si] // xg, splits[si] // xg, mt,
                                 first=(si == 0))
                    span = (s_start[si], s_start[si] + splits[si])
                    if si == 0:
                        outsb[mi] = op.tile(
                            [P, ns], f32, tag="outsb", name=f"outsb_0_{mi}"
                        )
                    for nstart, sz in chunks:
                        pst = ps.tile([P, MM_FREE], f32, tag="psum")
                        mm_sweep(pst, (nstart, sz), span[0], splits[si], [xbf], [span])
                        osl = outsb[mi][:, nstart:nstart + sz]
                        if si == 0:
                            nc.vector.tensor_tensor(
                                osl, pst[:, :sz], bias_exp[:, nstart:nstart + sz], ADD
                            )
                        else:
                            nc.vector.tensor_tensor(osl, osl, pst[:, :sz], ADD)
                    if mi >= start_mi:
                        j = mi - start_mi
                        for i in pending[j * per_mi:(j + 1) * per_mi]:
                            load_pair(i)
                    if si == s_n - 1:
                        nc.scalar.dma_start(
                            out.ap()[mt * P:(mt + 1) * P, :], outsb[mi][:]
                        )

            # ---- blocks 1+: full-k accumulation spans ----
            half = kt_n // 2
            for mt in range(mt_block, mt_n):
                slabs = [x_slab(0, half // xg, mt), x_slab(half // xg, half // xg, mt)]
                slab_kts = [(0, half), (half, kt_n)]
                osb = op.tile([P, ns], f32, tag="outsb", name=f"outsb_{mt}")
                for nstart, sz in chunks:
                    pst = ps.tile([P, MM_FREE], f32, tag="psum")
                    mm_sweep(pst, (nstart, sz), 0, kt_n, slabs, slab_kts)
                    nc.vector.tensor_tensor(
                        osb[:, nstart:nstart + sz],
                        pst[:, :sz],
                        bias_exp[:, nstart:nstart + sz],
                        ADD,
                    )
                    if mt == mt_n - 1:
                        # last m-tile: ship each chunk as soon as it lands so
                        # the kernel tail isn't one big serial DMA
                        nc.scalar.dma_start(
                            out.ap()[mt * P:(mt + 1) * P, nstart:nstart + sz],
                            osb[:, nstart:nstart + sz],
                        )
                if mt != mt_n - 1:
                    nc.scalar.dma_start(out.ap()[mt * P:(mt + 1) * P, :], osb[:])

    nc.compile()
    return nc


def prep_x(x, xg=XG):
    """bf16 x^T swizzled so each (kg, mt) slab is one contiguous [128, xg*128]
    row-block: xt[(kg*mt_n + mt)*128 + p, kl*128 + j] = x[mt*128 + j, (kg*xg + kl)*128 + p]
    """
    m, k = x.shape
    kt_n, mt_n = k // P, m // P
    kg_n = kt_n // xg
    xbf = x.astype(ml_dtypes.bfloat16)
    # [mt, j, kg, kl, p] -> [kg, mt, p, kl, j]
    xs = xbf.reshape(mt_n, P, kg_n, xg, P).transpose(2, 0, 4, 3, 1)
    return np.ascontiguousarray(xs.reshape(kg_n * mt_n * P, xg * P))


def prep_inputs(x, qweight, qscales, qzeros, bias):
    """Host-side shard/layout prep. Returns per-core input maps."""
    x = np.asarray(x)
    qweight = np.asarray(qweight)
    qscales = np.asarray(qscales).astype(np.float32)
    qzeros = np.asarray(qzeros).astype(np.float32)
    bias = np.asarray(bias)

    xprep = prep_x(x)

    # Unpack int4 nibbles into k-major fp32 [K, N], dequantize per group,
    # round once to bf16: even k -> low nibble, odd k -> high nibble.
    b = qweight.astype(np.uint8)              # [N, K//2]
    q4 = np.empty((K, NFULL), np.float32)
    q4[0::2, :] = (b & 15).T
    q4[1::2, :] = (b >> 4).T
    kt_n = K // P
    # per-group affine along k: group g covers rows [g*128, (g+1)*128)
    q4 = q4.reshape(kt_n, P, NFULL)
    q4 -= qzeros.T[:, None, :]
    q4 *= qscales.T[:, None, :]
    wbf = q4.astype(ml_dtypes.bfloat16)       # [kt, p, N]
    # partition-major: wdp[p, kt, n] = W^T[kt*128 + p, n]
    wdp = np.ascontiguousarray(wbf.transpose(1, 0, 2))

    bias2d = bias.astype(ml_dtypes.bfloat16).reshape(1, NFULL)

    in_maps = []
    for c in range(NCORES):
        sl = slice(c * NS, (c + 1) * NS)
        in_maps.append(
            {
                "xt": xprep,
                "wd": np.ascontiguousarray(wdp[:, :, sl]).reshape(P, kt_n * NS),
                "bias": np.ascontiguousarray(bias2d[:, sl]),
            }
        )
    return in_maps


def _run(in_maps, trace):
    global LAST_RESULTS
    from concourse.bass_utils import run_bass_kernel_spmd

    nc = build_nc()
    res = run_bass_kernel_spmd(
        nc,
        in_maps,
        core_ids=list(range(NCORES)),
        trace=trace,
        trace_cores=list(range(NCORES)) if trace else None,
    )
    LAST_RESULTS = res
    return np.concatenate([res.results[c]["out"] for c in range(NCORES)], axis=1)


def _subprocess_entry(tmpdir):
    """Retry entry point: runs the kernel in a fresh process (clean device
    state) using inputs staged in tmpdir, writes the output there."""
    import numpy as np  # noqa: F811 (fresh interpreter)

    data = np.load(os.path.join(tmpdir, "inputs.npz"))
    out = kernel(**{k: data[k] for k in data.files})
    np.save(os.path.join(tmpdir, "out.npy"), out)


def kernel(x, qweight, qscales, qzeros, bias):
    """Full-input entry: shard, run on 8 cores, gather. Retries on transient
    device failures (NRT exec-unit errors have been observed sporadically on
    this fabric): once in-process with NEURON_RT_RESET_CORES=1, then once in
    a fresh subprocess."""
    in_maps = prep_inputs(x, qweight, qscales, qzeros, bias)
    trace = bool(os.environ.get("BASS_AWQ_TRACE"))
    try:
        return _run(in_maps, trace)
    except Exception:
        pass
    os.environ["NEURON_RT_RESET_CORES"] = "1"
    try:
        return _run(in_maps, trace)
    except Exception:
        if os.environ.get("AWQ_NO_SUBPROC"):
            raise
    # last resort: fresh process (clean runtime/device handles)
    import subprocess
    import sys
    import tempfile

    with tempfile.TemporaryDirectory() as td:
        np.savez(
            os.path.join(td, "inputs.npz"),
            x=x, qweight=qweight, qscales=qscales, qzeros=qzeros, bias=bias,
        )
        code = (
            "import importlib.util, sys;"
            f"spec = importlib.util.spec_from_file_location('awq_kernel', {__file__!r});"
            "m = importlib.util.module_from_spec(spec);"
            "spec.loader.exec_module(m);"
            f"m._subprocess_entry({td!r})"
        )
        subprocess.run(
            [sys.executable, "-c", code],
            check=True,
            env={**os.environ, "NEURON_RT_RESET_CORES": "1",
                 "AWQ_NO_SUBPROC": "1"},
        )
        return np.load(os.path.join(td, "out.npy"))


# revision 27
# speedup vs baseline: 1.0160x; 1.0064x over previous
"""AWQ linear (int4 group-quantized) matmul on 8 Trainium2 NeuronCores.

out[m, n] = sum_k x[m, k] * W[n, k] + bias[n]
W[n, k] = (q4[n, k] - qzeros[n, k//128]) * qscales[n, k//128]

Column-parallel: shard N=11008 across 8 cores (1376 each), replicate x.
The weights are dequantized to bf16 on the host (same DMA bytes as the
previous device-side scheme, which already shipped the nibbles as bf16)
and streamed straight into resident SBUF, so the PE starts as soon as
the first k-tiles land and the DVE only does psum evictions:
  - host computes W^T bf16 [K, N], partition-major per k-tile, plus the
    x^T swizzle so every (k-group, m-tile) slab is one contiguous DMA
  - m-tile block 0 accumulates over k in splits of 2/2/4/8/16 k-tiles
    so the PE starts after the first 2 W k-tiles (~2us of DMA); later
    splits' W loads are issued on the Sync queue behind the x slabs so
    x-pool backpressure paces them just-in-time (HBM reads are the
    scarce resource while W streams in)
  - the remaining m-tiles run one full-k accumulation per psum chunk
  - bias is fused into the first eviction; the last m-tile ships its
    output per-chunk to shorten the kernel tail
"""

import os

import numpy as np
import ml_dtypes

M, K, NFULL = 4096, 4096, 11008
NCORES = 8
NS = NFULL // NCORES          # 1376 out-features per core
P = 128                       # partitions; also the quant group size
MM_FREE = 512                 # psum bank limit (fp32)
XG = 2                        # k-tiles per x-slab group in the host layout

LAST_RESULTS = None           # BassKernelResults of the last kernel() call


def build_nc(k=K, m=M, ns=NS, n_cores=NCORES, splits=(2, 2, 4, 8, 16), mt_block=8, xg=XG):
    """Build + compile the per-core Bass program (SPMD: same NEFF on all cores).

    Block 0 accumulates over k in `splits` (so the PE can start while later
    k-groups' weights are still in flight); the remaining blocks run one
    full-k span.
    """
    import concourse.bass as bass
    import concourse.mybir as mybir
    import concourse.tile as tile
    from concourse import bacc

    kt_n = k // P
    mt_n = m // P
    assert sum(splits) == kt_n and mt_block <= mt_n
    assert all(s % xg == 0 for s in splits) and all(s % 2 == 0 for s in splits)
    chunks = [(i, min(MM_FREE, ns - i)) for i in range(0, ns, MM_FREE)]

    f32 = mybir.dt.float32
    bf16 = mybir.dt.bfloat16
    ADD = mybir.AluOpType.add

    nc = bacc.Bacc("TRN2", num_devices=n_cores)
    # xt rows are (kg, mt, p): each (kg, mt) slab is contiguous [128, xg*128]
    xt = nc.dram_tensor("xt", [(kt_n // xg) * mt_n * P, xg * P], bf16, kind="ExternalInput")
    # host-dequantized W^T, rows are partitions: wd[p, kt*ns + n] = W^T[kt*128 + p, n]
    wd = nc.dram_tensor("wd", [P, kt_n * ns], bf16, kind="ExternalInput")
    # bias ships bf16 (halves the partition-broadcast bytes competing with the
    # critical first W k-tile); converted once to fp32 on-chip
    bias = nc.dram_tensor("bias", [1, ns], bf16, kind="ExternalInput")
    out = nc.dram_tensor("out", [m, ns], f32, kind="ExternalOutput")

    with tile.TileContext(nc) as tc:
        with (
            tc.tile_pool(name="persist", bufs=1) as persist,
            tc.tile_pool(name="xp0", bufs=3) as xp0,
            tc.tile_pool(name="xp1", bufs=4) as xp1,
            tc.tile_pool(name="xp2", bufs=4) as xp2,
            tc.tile_pool(name="op", bufs=mt_block + 1) as op,
            tc.tile_pool(name="ps", bufs=8, space="PSUM") as ps,
        ):
            w_all = persist.tile([P, kt_n, ns], bf16)
            bias_exp = persist.tile([P, ns], f32)
            bias_bf = persist.tile([P, ns], bf16)

            def load_pair(i, eng=None, fine=False):
                """DMA W k-tiles 2i and 2i+1 into resident SBUF (one load
                covers both tiles -- contiguous per partition). Early pairs
                ride the SWDGE ring; pairs interleaved into block 0 ride the
                Sync queue BEHIND the x slabs, so x-pool backpressure paces
                the W stream just-in-time instead of letting it saturate HBM
                reads and starve the x path."""
                eng = eng or nc.sync
                kt = 2 * i
                if i == 0:
                    # pair 0 gates the first sweeps: load per (k-tile,
                    # n-chunk) in sweep order, k-tile 0 on the SWDGE ring and
                    # k-tile 1 on the ACT ring, so the very first matmul
                    # waits on a single 128KB transfer
                    for n0, szc in chunks:
                        for j, eng_j in ((0, nc.gpsimd), (1, nc.scalar)):
                            srcap = bass.AP(
                                wd.ap().tensor,
                                j * ns + n0,
                                [[kt_n * ns, P], [1, szc]],
                            )
                            eng_j.dma_start(w_all[:, j, n0:n0 + szc], srcap)
                    return
                if fine:
                    # kt-granular loads: an x slab queued behind W then waits
                    # ~1us instead of ~2us (transfers fair-share the ring)
                    for j in range(2):
                        srcj = wd.ap()[:, (kt + j) * ns:(kt + j + 1) * ns]
                        eng.dma_start(
                            w_all[:, kt + j:kt + j + 1, :],
                            srcj.rearrange("p (j n) -> p j n", n=ns),
                        )
                    return
                src = wd.ap()[:, kt * ns:(kt + 2) * ns]
                eng.dma_start(
                    w_all[:, kt:kt + 2, :],
                    src.rearrange("p (j n) -> p j n", n=ns),
                )

            def x_slab(g0, ng, mt, first=False):
                """Load x k-groups g0..g0+ng-1 for m-tile mt: [128, ng, xg*128],
                as ONE strided DMA (one issue slot on the Sync queue). si=0
                uses a dedicated depth-2 pool: its third slab (and the W pair
                queued behind it) then can't issue until the PE is running,
                keeping HBM clear for the startup-critical transfers."""
                pool = xp0 if first else (xp1 if ng == 1 else xp2)
                xbf = pool.tile([P, ng, xg * P], bf16,
                                tag=f"xbf{ng}a" if first else f"xbf{ng}")
                row = xg * P
                base = (g0 * mt_n + mt) * P
                src = bass.AP(
                    xt.ap().tensor,
                    base * row,
                    [[row, P], [mt_n * P * row, ng], [1, row]],
                )
                nc.sync.dma_start(xbf[:], src)
                return xbf

            def mm_sweep(pst, sz_args, kt0, n_kt, slabs, slab_kts):
                """Accumulate kt0..kt0+n_kt-1 into pst from the given x slabs."""
                nstart, sz = sz_args
                for kl in range(n_kt):
                    kt = kt0 + kl
                    sb_i = next(i for i, (a, b) in enumerate(slab_kts) if a <= kt < b)
                    loc = kt - slab_kts[sb_i][0]
                    nc.tensor.matmul(
                        pst[:, :sz],
                        slabs[sb_i][:, loc // xg, (loc % xg) * P:(loc % xg + 1) * P],
                        w_all[:, kt, nstart:nstart + sz],
                        start=(kl == 0),
                        stop=(kl == n_kt - 1),
                    )

            for i in range(splits[0] // 2):
                load_pair(i, eng=nc.gpsimd)
            # bias rides the ACT queue behind pair 0's k-tile-1 chunks: it is
            # only needed at the first eviction, which the psum pool lets lag
            nc.scalar.dma_start(bias_bf[:], bias.ap().to_broadcast((P, ns)))
            nc.vector.tensor_scalar_add(bias_exp[:], bias_bf[:], 0.0)

            s_n = len(splits)
            s_start = [sum(splits[:i]) for i in range(s_n)]

            # ---- block 0: k-split sweeps, W loads interleaved ----
            outsb = {}
            for si in range(s_n):
                pending = (
                    list(range(s_start[si + 1] // 2,
                               (s_start[si + 1] + splits[si + 1]) // 2))
                    if si + 1 < s_n
                    else []
                )
                # si=0: hold pending W issues until mi=2 -- behind the first
                # pool-gated x slab, so they can't issue before the PE runs
                start_mi = 2 if si == 0 else 0
                navail = mt_block - start_mi
                per_mi = (len(pending) + navail - 1) // navail if pending else 0
                for mi in range(mt_block):
                    mt = mi
                    xbf = x_slab(s_start[si] // xg, splits[si] // xg, mt,
                                 first=(si == 0))
                    span = (s_start[si], s_start[si] + splits[si])
                    if si == 0:
                        outsb[mi] = op.tile(
                            [P, ns], f32, tag="outsb", name=f"outsb_0_{mi}"
                        )
                    for nstart, sz in chunks:
                        pst = ps.tile([P, MM_FREE], f32, tag="psum")
                        mm_sweep(pst, (nstart, sz), span[0], splits[si], [xbf], [span])
                        osl = outsb[mi][:, nstart:nstart + sz]
                        if si == 0:
                            nc.vector.tensor_tensor(
                                osl, pst[:, :sz], bias_exp[:, nstart:nstart + sz], ADD
                            )
                        else:
                            nc.vector.tensor_tensor(osl, osl, pst[:, :sz], ADD)
                    if mi >= start_mi:
                        j = mi - start_mi
                        for i in pending[j * per_mi:(j + 1) * per_mi]:
                            load_pair(i, fine=(si >= 2))
                    if si == s_n - 1:
                        nc.scalar.dma_start(
                            out.ap()[mt * P:(mt + 1) * P, :], outsb[mi][:]
                        )

            # ---- blocks 1+: full-k accumulation spans ----
            half = kt_n // 2
            for mt in range(mt_block, mt_n):
                slabs = [x_slab(0, half // xg, mt), x_slab(half // xg, half // xg, mt)]
                slab_kts = [(0, half), (half, kt_n)]
                osb = op.tile([P, ns], f32, tag="outsb", name=f"outsb_{mt}")
                for nstart, sz in chunks:
                    pst = ps.tile([P, MM_FREE], f32, tag="psum")
                    mm_sweep(pst, (nstart, sz), 0, kt_n, slabs, slab_kts)
                    nc.vector.tensor_tensor(
                        osb[:, nstart:nstart + sz],
                        pst[:, :sz],
                        bias_exp[:, nstart:nstart + sz],
                        ADD,
                    )
                    if mt == mt_n - 1:
                        # last m-tile: ship each chunk as soon as it lands so
                        # the kernel tail isn't one big serial DMA
                        nc.scalar.dma_start(
                            out.ap()[mt * P:(mt + 1) * P, nstart:nstart + sz],
                            osb[:, nstart:nstart + sz],
                        )
                if mt != mt_n - 1:
                    nc.scalar.dma_start(out.ap()[mt * P:(mt + 1) * P, :], osb[:])

    nc.compile()
    return nc


def prep_x(x, xg=XG):
    """bf16 x^T swizzled so each (kg, mt) slab is one contiguous [128, xg*128]
    row-block: xt[(kg*mt_n + mt)*128 + p, kl*128 + j] = x[mt*128 + j, (kg*xg + kl)*128 + p]
    """
    m, k = x.shape
    kt_n, mt_n = k // P, m // P
    kg_n = kt_n // xg
    xbf = x.astype(ml_dtypes.bfloat16)
    # [mt, j, kg, kl, p] -> [kg, mt, p, kl, j]
    xs = xbf.reshape(mt_n, P, kg_n, xg, P).transpose(2, 0, 4, 3, 1)
    return np.ascontiguousarray(xs.reshape(kg_n * mt_n * P, xg * P))


def prep_inputs(x, qweight, qscales, qzeros, bias):
    """Host-side shard/layout prep. Returns per-core input maps."""
    x = np.asarray(x)
    qweight = np.asarray(qweight)
    qscales = np.asarray(qscales).astype(np.float32)
    qzeros = np.asarray(qzeros).astype(np.float32)
    bias = np.asarray(bias)

    xprep = prep_x(x)

    # Unpack int4 nibbles into k-major fp32 [K, N], dequantize per group,
    # round once to bf16: even k -> low nibble, odd k -> high nibble.
    b = qweight.astype(np.uint8)              # [N, K//2]
    q4 = np.empty((K, NFULL), np.float32)
    q4[0::2, :] = (b & 15).T
    q4[1::2, :] = (b >> 4).T
    kt_n = K // P
    # per-group affine along k: group g covers rows [g*128, (g+1)*128)
    q4 = q4.reshape(kt_n, P, NFULL)
    q4 -= qzeros.T[:, None, :]
    q4 *= qscales.T[:, None, :]
    wbf = q4.astype(ml_dtypes.bfloat16)       # [kt, p, N]
    # partition-major: wdp[p, kt, n] = W^T[kt*128 + p, n]
    wdp = np.ascontiguousarray(wbf.transpose(1, 0, 2))

    bias2d = bias.astype(ml_dtypes.bfloat16).reshape(1, NFULL)

    in_maps = []
    for c in range(NCORES):
        sl = slice(c * NS, (c + 1) * NS)
        in_maps.append(
            {
                "xt": xprep,
                "wd": np.ascontiguousarray(wdp[:, :, sl]).reshape(P, kt_n * NS),
                "bias": np.ascontiguousarray(bias2d[:, sl]),
            }
        )
    return in_maps


def _run(in_maps, trace):
    global LAST_RESULTS
    from concourse.bass_utils import run_bass_kernel_spmd

    nc = build_nc()
    res = run_bass_kernel_spmd(
        nc,
        in_maps,
        core_ids=list(range(NCORES)),
        trace=trace,
        trace_cores=list(range(NCORES)) if trace else None,
    )
    LAST_RESULTS = res
    return np.concatenate([res.results[c]["out"] for c in range(NCORES)], axis=1)


def _subprocess_entry(tmpdir):
    """Retry entry point: runs the kernel in a fresh process (clean device
    state) using inputs staged in tmpdir, writes the output there."""
    import numpy as np  # noqa: F811 (fresh interpreter)

    data = np.load(os.path.join(tmpdir, "inputs.npz"))
    out = kernel(**{k: data[k] for k in data.files})
    np.save(os.path.join(tmpdir, "out.npy"), out)


def kernel(x, qweight, qscales, qzeros, bias):
    """Full-input entry: shard, run on 8 cores, gather. Retries on transient
    device failures (NRT exec-unit errors have been observed sporadically on
    this fabric): once in-process with NEURON_RT_RESET_CORES=1, then once in
    a fresh subprocess."""
    in_maps = prep_inputs(x, qweight, qscales, qzeros, bias)
    trace = bool(os.environ.get("BASS_AWQ_TRACE"))
    try:
        return _run(in_maps, trace)
    except Exception:
        pass
    os.environ["NEURON_RT_RESET_CORES"] = "1"
    try:
        return _run(in_maps, trace)
    except Exception:
        if os.environ.get("AWQ_NO_SUBPROC"):
            raise
    # last resort: fresh process (clean runtime/device handles)
    import subprocess
    import sys
    import tempfile

    with tempfile.TemporaryDirectory() as td:
        np.savez(
            os.path.join(td, "inputs.npz"),
            x=x, qweight=qweight, qscales=qscales, qzeros=qzeros, bias=bias,
        )
        code = (
            "import importlib.util, sys;"
            f"spec = importlib.util.spec_from_file_location('awq_kernel', {__file__!r});"
            "m = importlib.util.module_from_spec(spec);"
            "spec.loader.exec_module(m);"
            f"m._subprocess_entry({td!r})"
        )
        subprocess.run(
            [sys.executable, "-c", code],
            check=True,
            env={**os.environ, "NEURON_RT_RESET_CORES": "1",
                 "AWQ_NO_SUBPROC": "1"},
        )
        return np.load(os.path.join(td, "out.npy"))
